# revision 2
# baseline (speedup 1.0000x reference)
"""Trainium2 Bass kernel: depthwise 3x3 stencil conv (SAME, zero-pad) + residual.

Math (per image, per channel):
    out[h,w] = sum_{dh,dw} k[dh,dw] * x[h+dh-1, w+dw-1]  +  x[h,w]

The fixed stencil k = [[1,0,-1],[0,1,0],[-1,0,1]] is rank-2:
    k = outer((1,0,-1),(1,0,-1)) + center(1)
so with t[h,w] = x[h-1,w] - x[h+1,w] (vertical pass):
    out[h,w] = 2*x[h,w] + t[h,w-1] - t[h,w+1]

Device computes out/2 = (beta/2)*x + t'[w-1] - t'[w+1] with t' = (V/2)^T @ x,
all bf16; host upconverts and multiplies by 2 (exact).

Layout: host packs each core's 4 images h-major: x_d[h, i*10752 + w*96 + ch]
([112, 43008] bf16).  One SBUF slab xs holds x, is updated in place
(x -> v -> out/2) and is the store source; ts holds t' with 96-col zero
halos per image block (10944 cols each).

Engine pipeline per image (21 matmul chunks of 512 cols, drained in
groups of <=2048 cols through 2 ping-pong 4-bank PSUM tiles):
    SP  ring: input loads (half-image granularity, image0 split finer)
    PE      : t' = vt^T @ xs chunk into PSUM, sem inc per drain group
    ACT     : PSUM -> ts bf16 copies (1x engine, ~9.9us/image)
              + issues output stores on the second HWDGE ring
    DVE     : op1 v = (beta/2) x + t'[w-1]; op2 out/2 = v - t'[w+1]
              (bf16 tensor_tensor, 2x_1P mode, in place over xs)

Key DMA fact (measured): one HWDGE ring sustains only ~283 GB/s/core on
loads (HBM read latency exposed), but with loads on the SP ring and
stores on the ACT ring the SDMA engines alternate packets from both
queues at ~430 GB/s/core aggregate.  So stores are issued from ACT as
soon as each output piece is ready, interleaved with the drain copies in
an order that keeps ACT from stalling on DVE.
"""

import sys
import numpy as np

for _p in ("/opt/trn_rl_repo",):
    if _p not in sys.path:
        sys.path.insert(0, _p)

# ---------------- problem constants (hardcoded per contract) ----------------
N_CORES = 8
N, H, W, CH = 32, 112, 112, 96
IMGS = N // N_CORES                    # 4 images per core
C = W * CH                             # 10752 cols per image
COLS = IMGS * C                        # 43008 cols per core slab
PAD = CH                               # one w column = 96 cols
TSB = C + 2 * PAD                      # 10944 cols per ts image block
MM_N = 512                             # matmul chunk (one PSUM bank of fp32)
HALF = C // 2                          # 5376

# drain groups per image: 6 groups of (2048 x 5, 512) cols
GRP = [(k * 2048, 2048) for k in range(5)] + [(10240, 512)]
NG = len(GRP)                          # 6

_CACHE = {}
LAST_RESULTS = None  # BassKernelResults of the most recent run (for test.py)


def _build_bass(beta):
    from concourse import bass, mybir

    bf16 = mybir.dt.bfloat16
    f32 = mybir.dt.float32
    nc = bass.Bass(debug=False)
    x_d = nc.declare_dram_parameter("x", [H, COLS], bf16, isOutput=False)
    v_d = nc.declare_dram_parameter("vmat", [H, H], bf16, isOutput=False)
    out_d = nc.declare_dram_parameter("out", [H, COLS], bf16, isOutput=True)
    warm_d = nc.declare_dram_parameter("warm", [H, 64], bf16, isOutput=True)

    vt = nc.alloc_sbuf_tensor("vt", [H, H], bf16)
    xs = nc.alloc_sbuf_tensor("xs", [H, COLS], bf16)
    ts = nc.alloc_sbuf_tensor("ts", [H, IMGS * TSB], bf16)
    scr = nc.alloc_sbuf_tensor("scr", [H, 128], bf16)
    ps = [nc.alloc_psum_tensor(f"ps{b}", [H, 2048], f32) for b in range(2)]

    # input loads (col ranges of x_d/xs); image 0 split finer so PE starts early
    LOADS = [(0, 2048), (2048, 5376), (5376, 10752)]
    for i in range(1, IMGS):
        LOADS.append((i * C, i * C + HALF))
        LOADS.append((i * C + HALF, (i + 1) * C))

    # DVE inc schedule -> stores.  image0: 4 pieces; 1-2: halves; 3: 2+2 tail
    # stores: (image, col0, len, dve_count_required)
    STORES = [
        (0, 0, 2048, 1), (0, 2048, 2048, 2), (0, 4096, 2048, 3), (0, 6144, 4608, 4),
        (1, 0, HALF, 5), (1, HALF, HALF, 6),
        (2, 0, HALF, 7), (2, HALF, HALF, 8),
        (3, 0, HALF, 9), (3, HALF, 2688, 10), (3, HALF + 2688, 2688, 11),
    ]
    N_ST = len(STORES) + 1  # + warm store

    def grp_of_interior(icol):
        # drain group index (within image) containing t' interior col icol
        return min(icol // 2048, NG - 1) if icol >= 0 else 0

    from contextlib import ExitStack

    with (
        nc.Block(no_gpsimd_drain=True) as block,
        nc.semaphore("s_vt") as s_vt,
        nc.semaphore("s_pe") as s_pe,
        nc.semaphore("s_act") as s_act,
        nc.semaphore("s_dve") as s_dve,
        nc.semaphore("s_st") as s_st,
        ExitStack() as _sems,
    ):
        s_ld = [
            _sems.enter_context(nc.semaphore(f"s_ld{u}")) for u in range(len(LOADS))
        ]

        @block.sync
        def _(sp: bass.BassEngine):
            a0, b0 = LOADS[0]
            sp.dma_start(out=xs[:, a0:b0], in_=x_d[:, a0:b0]).then_inc(s_ld[0], 16)
            sp.dma_start(out=vt[:, :], in_=v_d[:, :]).then_inc(s_vt, 16)
            for u, (a, b) in enumerate(LOADS):
                if u == 0:
                    continue
                sp.dma_start(out=xs[:, a:b], in_=x_d[:, a:b]).then_inc(s_ld[u], 16)
            sp.wait_ge(s_st, 16 * N_ST)

        @block.tensor
        def _(pe: bass.BassEngine):
            pe.wait_ge(s_vt, 16)
            waited = [False] * len(LOADS)

            def need_cols(hi):
                for u, (a, b) in enumerate(LOADS):
                    if a < hi and not waited[u]:
                        pe.wait_ge(s_ld[u], 16)
                        waited[u] = True

            for gg in range(IMGS * NG):
                i, k = divmod(gg, NG)
                g0, gsz = GRP[k]
                base = i * C + g0
                need_cols(base + gsz)
                if gg >= 2:
                    pe.wait_ge(s_act, gg - 1)
                nch = gsz // MM_N
                for c in range(nch):
                    mm = pe.matmul(
                        out=ps[gg % 2][0:H, c * MM_N : (c + 1) * MM_N],
                        lhsT=vt[:, :],
                        rhs=xs[:, base + c * MM_N : base + (c + 1) * MM_N],
                        start=True,
                        stop=True,
                    )
                    if c == nch - 1:
                        mm.then_inc(s_pe, 1)

        @block.scalar
        def _(act: bass.BassEngine):
            # warm the ACT HWDGE ring + the activation table during preamble
            act.dma_start(out=warm_d[:, :], in_=scr[:, 0:64]).then_inc(s_st, 16)
            act.mul(scr[:, 64:66], ps[0][0:H, 0:2], 0.5)

            st_iter = iter(STORES)
            pending = next(st_iter)

            def flush_stores(upto_dve):
                # emit stores whose dve requirement is <= upto_dve
                nonlocal pending
                while pending is not None and pending[3] <= upto_dve:
                    i, c0, ln, req = pending
                    act.wait_ge(s_dve, req)
                    act.dma_start(
                        out=out_d[:, i * C + c0 : i * C + c0 + ln],
                        in_=xs[:, i * C + c0 : i * C + c0 + ln],
                    ).then_inc(s_st, 16)
                    pending = next(st_iter, None)

            # dve counts available "by now" if ACT emits stores right after
            # the drain group that unblocks the corresponding DVE ops.
            # schedule[gg] = dve count safe to wait for after drain gg.
            # image0 dve incs 1..4 happen after drains 2,3,4,6; images i>=1
            # inc 2i+3 after drain 6i+3ish, 2i+4 after 6(i+1).  Emit lagged:
            DVE_AFTER = {2: 0, 3: 1, 4: 2, 5: 3, 8: 4, 11: 5, 14: 6, 17: 7, 20: 8, 23: 9}
            for gg in range(IMGS * NG):
                i, k = divmod(gg, NG)
                g0, gsz = GRP[k]
                act.wait_ge(s_pe, gg + 1)
                act.copy(
                    ts[:, i * TSB + PAD + g0 : i * TSB + PAD + g0 + gsz],
                    ps[gg % 2][0:H, 0:gsz],
                ).then_inc(s_act, 1)
                if gg in DVE_AFTER:
                    flush_stores(DVE_AFTER[gg])
            flush_stores(99)

        @block.vector
        def _(dve: bass.BassEngine):
            # zero the per-image halo slivers of ts once (DVE-local ordering)
            for i in range(IMGS):
                dve.memset(ts[:, i * TSB : i * TSB + PAD], 0.0)
                dve.memset(ts[:, i * TSB + PAD + C : (i + 1) * TSB], 0.0)

            def op1(i, lo, hi):
                # v = (beta/2)*x + t'[w-1]  over image-i interior [lo, hi)
                if beta == 2.0:
                    dve.tensor_tensor(
                        out=xs[:, i * C + lo : i * C + hi],
                        in0=xs[:, i * C + lo : i * C + hi],
                        in1=ts[:, i * TSB + lo : i * TSB + hi],
                        op=mybir.AluOpType.add,
                    )
                else:
                    dve.scalar_tensor_tensor(
                        out=xs[:, i * C + lo : i * C + hi],
                        in0=xs[:, i * C + lo : i * C + hi],
                        scalar=float(beta) / 2.0,
                        in1=ts[:, i * TSB + lo : i * TSB + hi],
                        op0=mybir.AluOpType.mult,
                        op1=mybir.AluOpType.add,
                    )

            def op2(i, lo, hi):
                dve.tensor_tensor(
                    out=xs[:, i * C + lo : i * C + hi],
                    in0=xs[:, i * C + lo : i * C + hi],
                    in1=ts[:, i * TSB + 2 * PAD + lo : i * TSB + 2 * PAD + hi],
                    op=mybir.AluOpType.subtract,
                )

            def wait_grp(i, k):
                dve.wait_ge(s_act, i * NG + k + 1)

            # image 0: fine-grained pieces so the first stores launch early.
            # op1 piece [lo,hi) needs group of interior hi-96; op2 needs hi+96
            wait_grp(0, 1)
            op1(0, 0, 2048)
            op1(0, 2048, 4096)   # needs grp((4096-96))=1 ok
            op2(0, 0, 2048)      # needs grp(2048+96)=1 ok
            dve.drain().then_inc(s_dve, 1)
            wait_grp(0, 2)
            op1(0, 4096, 6144)
            op2(0, 2048, 4096)
            dve.drain().then_inc(s_dve, 1)
            wait_grp(0, 3)
            op1(0, 6144, 8192)
            op2(0, 4096, 6144)
            dve.drain().then_inc(s_dve, 1)
            wait_grp(0, 5)
            op1(0, 8192, C)
            op2(0, 6144, 8192)
            op2(0, 8192, C)
            dve.drain().then_inc(s_dve, 1)
            # images 1..3: halves
            for i in range(1, IMGS):
                wait_grp(i, 2)
                op1(i, 0, HALF)
                op2(i, 0, HALF)
                dve.drain().then_inc(s_dve, 1)
                wait_grp(i, 5)
                op1(i, HALF, C)
                if i < IMGS - 1:
                    op2(i, HALF, C)
                    dve.drain().then_inc(s_dve, 1)
                else:
                    op2(i, HALF, HALF + 2688)
                    dve.drain().then_inc(s_dve, 1)
                    op2(i, HALF + 2688, C)
                    dve.drain().then_inc(s_dve, 1)

    return nc


def _stencil_params(kern):
    """Validate the depthwise kernel and extract (vertical profile a, beta)."""
    k = np.asarray(kern, dtype=np.float32)
    if k.ndim != 4 or k.shape != (3, 3, 1, CH):
        return None
    if not np.all(k == k[:, :, :, :1]):
        return None
    k2 = k[:, :, 0, 0]
    if not (np.all(k2[:, 2] == -k2[:, 0]) and k2[0, 1] == 0 and k2[2, 1] == 0):
        return None
    return k2[:, 0].copy(), float(k2[1, 1]) + 1.0


def _numpy_fallback(x, kern):
    """Straightforward shifted-add implementation (safety net only)."""
    k = np.asarray(kern, dtype=np.float32)[:, :, 0, :]  # (3,3,CH)
    xp = np.pad(x, ((0, 0), (1, 1), (1, 1), (0, 0)))
    out = x.astype(np.float32).copy()
    for dh in range(3):
        for dw in range(3):
            out += k[dh, dw] * xp[:, dh : dh + H, dw : dw + W, :]
    return out


def _ensure_ntff_hook():
    """The agent image's antenv lacks axon_hooks; synthesize it so
    run_bass_kernel_spmd(trace=True) can reach the NTFF profiler."""
    import types

    if "antenv.axon_hooks" in sys.modules:
        return
    import antenv

    mod = types.ModuleType("antenv.axon_hooks")
    state = {}
    mod.set_axon_ntff_profile_hook = lambda h: state.__setitem__("h", h)
    mod.get_axon_ntff_profile_hook = lambda: state.get("h")
    sys.modules["antenv.axon_hooks"] = mod
    antenv.axon_hooks = mod
    try:
        if "/root/.axon_site" not in sys.path:
            sys.path.insert(0, "/root/.axon_site")
        from trn_agent_boot.trn_boot import _ntff_profile_via_ctypes

        hook = _ntff_profile_via_ctypes("/opt/axon/libaxon_pjrt.so")
        if hook is not None:
            mod.set_axon_ntff_profile_hook(hook)
    except Exception:
        pass


def _run_on_hw(x, a, beta, trace=False):
    global LAST_RESULTS
    if trace:
        _ensure_ntff_hook()
    import ml_dtypes
    from concourse.bass_utils import run_bass_kernel_spmd

    bf16 = ml_dtypes.bfloat16

    # vertical banded matrix scaled by 0.5: V[i, j] = coeff of x-row i in
    # t'-row j (t' = t/2)
    V = np.zeros((H, H), dtype=np.float32)
    idx = np.arange(H)
    V[idx[:-1] + 1, idx[:-1]] += a[2]
    V[idx, idx] += a[1]
    V[idx[1:] - 1, idx[1:]] += a[0]
    Vb = (0.5 * V).astype(bf16)

    key = (a.tobytes(), float(beta))
    if key not in _CACHE:
        _CACHE[key] = _build_bass(float(beta))
    nc = _CACHE[key]

    # h-major pack: core c gets [112, 4*10752] with images side by side
    xb = np.ascontiguousarray(
        x.reshape(N_CORES, IMGS, H, C).astype(bf16).transpose(0, 2, 1, 3)
    ).reshape(N_CORES, H, COLS)
    in_maps = [{"x": xb[c], "vmat": Vb} for c in range(N_CORES)]
    res = run_bass_kernel_spmd(nc, in_maps, list(range(N_CORES)), trace=trace)
    LAST_RESULTS = res
    # device returned out/2 in bf16 (h-major); unpack + x2 (exact in fp32)
    o = np.stack(
        [np.asarray(res.results[c]["out"]) for c in range(N_CORES)]
    )  # [8, 112, 43008] bf16
    out = (
        o.reshape(N_CORES, H, IMGS, C)
        .transpose(0, 2, 1, 3)
        .astype(np.float32)
        .reshape(N, H, W, CH)
    )
    out *= 2.0
    return out


def kernel(x, kernel=None, _trace=False, **_unused):
    x = np.ascontiguousarray(np.asarray(x, dtype=np.float32))
    assert x.shape == (N, H, W, CH), f"unexpected x shape {x.shape}"
    if kernel is None:
        base = np.array(
            [[1.0, 0.0, -1.0], [0.0, 1.0, 0.0], [-1.0, 0.0, 1.0]], dtype=np.float32
        )
        kernel = np.tile(base[:, :, None, None], (1, 1, 1, CH))
    params = _stencil_params(kernel)
    if params is None:
        return _numpy_fallback(x, kernel)
    a, beta = params
    return _run_on_hw(x, a, beta, trace=_trace)


if __name__ == "__main__":
    xs = np.random.randn(N, H, W, CH).astype(np.float32)
    out = kernel(xs)
    print(out.shape, out.dtype)
